# revision 1
# baseline (speedup 1.0000x reference)
"""Trainium2 Bass kernel for ConvMosaic: 3x3 conv (pad 1) where the weight set
depends on output position p%16 == w%16 (column phase).

Strategy (8 NeuronCores, SPMD):
  - Shard over (N, H): core k handles image k//2, row-half k%2 (128 rows).
  - Host pre-pads each core chunk to [C=32, 130, 258] (halo rows + zero cols).
  - On-chip: x staged as 3 row-shifted partition blocks [(di,c)=96, 32h, 258w]
    so one matmul contracts K=96=(di,c); the dj taps are free-dim offsets.
  - Per phase s: PSUM tile [64oc, 32h, 16w16]; 3 accumulating fp32r matmuls
    (F=512 -> full-rate fp32r); evacuate with alternating DVE/ACT strided
    copies into a contiguous [64, 32, 256] output staging tile; big DMAs out.
"""

import sys

import numpy as np

for _p in ("/opt/trn_rl_repo",):
    if _p not in sys.path:
        sys.path.insert(0, _p)

N, C, H, W = 4, 32, 256, 256
OC = 64
SPE = 16
KDIM = 96  # (di, c) contraction partitions
NCORES = 8
ROWS = H * N // NCORES  # 128 rows per core
GR = 32  # rows per group
GROUPS = ROWS // GR

_cache = {}


def build_nc():
    import bass_rust
    from concourse import bacc, bass, mybir, tile

    f32 = mybir.dt.float32
    f32r = mybir.dt.float32r

    nc = bacc.Bacc()
    xin = nc.dram_tensor("xin", [C, ROWS + 2, W + 2], f32r, kind="ExternalInput")
    wdr = nc.dram_tensor("w", [KDIM, SPE, 3, OC], f32r, kind="ExternalInput")
    out = nc.dram_tensor("out", [OC, ROWS, W], f32, kind="ExternalOutput")

    with tile.TileContext(nc) as tc:
        with (
            tc.tile_pool(name="wp", bufs=1) as wp,
            tc.tile_pool(name="xp", bufs=3) as xp,
            tc.tile_pool(name="op", bufs=2) as op,
            tc.tile_pool(name="ppA", bufs=2, space=bass.MemorySpace.PSUM) as ppA,
            tc.tile_pool(name="ppB", bufs=2, space=bass.MemorySpace.PSUM) as ppB,
        ):
            w_sb = wp.tile([KDIM, SPE, 3, OC], f32r)

            x3s = []
            for g in range(GROUPS):
                x3 = xp.tile([KDIM, GR, W + 2], f32r)
                # single DMA: src AP iterates (di, c, h, ww) with overlapping
                # row windows (di stride = 1 row) -> fills all 96 partitions
                src = xin[:, g * GR : g * GR + GR + 2, :]
                src = src.unsqueeze(0).broadcast_to([3, C, GR + 2, W + 2])
                src.ap = bass_rust.VecI64Pair(
                    [
                        [W + 2, 3],
                        [(ROWS + 2) * (W + 2), C],
                        [W + 2, GR],
                        [1, W + 2],
                    ]
                )
                if g == 0:
                    nc.sync.dma_start(x3[:], src)
                    nc.sync.dma_start(w_sb[:], wdr[:])
                    # dummy matmul: absorbs the weight-DMA wait so the first
                    # real matmul carries only one wait (self-loading fp32r
                    # matmuls support a single sync wait).
                    scratch = ppA.tile([OC, 16], f32, tag="ps")
                    nc.tensor.matmul(
                        scratch[:],
                        w_sb[:, 0, 0, :],
                        w_sb[:, 0, 0, 0:16],
                        start=True,
                        stop=True,
                    )
                else:
                    nc.sync.dma_start(x3[:], src)
                x3s.append(x3)

            for g in range(GROUPS):
                x3 = x3s[g]
                o_sb = op.tile([OC, GR, W], f32)
                pp = ppA if g % 2 == 0 else ppB
                for sp in range(SPE // 2):
                    # two phases (2*sp, 2*sp+1) share one 2-bank psum tile so
                    # the evacuation is a single wider DVE copy
                    ps = pp.tile([OC, 2, GR, 16], f32, tag="ps")
                    for b in range(2):
                        s = 2 * sp + b
                        for dj in range(3):
                            nc.tensor.matmul(
                                ps[:, b],
                                w_sb[:, s, dj, :],
                                x3[:, :, s + dj : s + dj + 241 : 16],
                                start=(dj == 0),
                                stop=(dj == 2),
                            )
                    # dst covers w columns {w16*16 + 2*sp + b}; iteration
                    # order (oc, b, h, w16) matches the contiguous psum src
                    dst = o_sb[:, :, 2 * sp : 2 * sp + 241 : 16]
                    dst = dst.unsqueeze(1).broadcast_to([OC, 2, GR, 16])
                    dst.ap = bass_rust.VecI64Pair(
                        [
                            [GR * W, OC],
                            [1, 2],
                            [W, GR],
                            [16, 16],
                        ]
                    )
                    nc.vector.tensor_copy(dst, ps[:])
                nc.scalar.dma_start(out[:, g * GR : (g + 1) * GR, :], o_sb[:])
    nc.finalize()
    return nc


def shard_inputs(x, weight):
    x = np.ascontiguousarray(np.asarray(x), dtype=np.float32)
    weight = np.ascontiguousarray(np.asarray(weight), dtype=np.float32)
    xp = np.zeros((N, C, H + 2, W + 2), np.float32)
    xp[:, :, 1:-1, 1:-1] = x
    # wh[(di*32+c), s, dj, oc] = weight[s, c*9+di*3+dj, oc]
    wh = np.ascontiguousarray(
        weight.reshape(SPE, C, 3, 3, OC).transpose(2, 1, 0, 3, 4)
    ).reshape(KDIM, SPE, 3, OC)
    in_maps = []
    for k in range(NCORES):
        n, r0 = k // 2, (k % 2) * ROWS
        in_maps.append(
            {"xin": np.ascontiguousarray(xp[n, :, r0 : r0 + ROWS + 2, :]), "w": wh}
        )
    return in_maps


def unshard_outputs(results):
    out = np.empty((N, OC, H, W), np.float32)
    for k in range(NCORES):
        n, r0 = k // 2, (k % 2) * ROWS
        out[n, :, r0 : r0 + ROWS, :] = results[k]["out"]
    return out


def run(x, weight, **spmd_kwargs):
    from concourse.bass_utils import run_bass_kernel_spmd

    in_maps = shard_inputs(x, weight)
    if "nc" not in _cache:
        _cache["nc"] = build_nc()
    res = run_bass_kernel_spmd(_cache["nc"], in_maps, list(range(NCORES)), **spmd_kwargs)
    return unshard_outputs(res.results), res


def kernel(x, weight):
    out, _ = run(x, weight)
    return out



# revision 3
# speedup vs baseline: 1.9936x; 1.9936x over previous
"""Trainium2 Bass kernel for ConvMosaic: 3x3 conv (pad 1) where the weight set
depends on output position p%16 == w%16 (column phase).

Strategy (8 NeuronCores, SPMD):
  - Shard over (N, H): core k handles image k//2, row-half k%2 (128 rows).
  - Host pre-pads each core chunk to [C=32, 130, 258] bf16 (halo rows + zero
    cols).  bf16 halves DMA traffic and doubles PE column rate.
  - On-chip: x staged as [(c,di)=96, 32h, 258w] tiles (c-major partition
    order so the staging DMA's outermost AP dim is 32 -> 16-engine fanout);
    one matmul contracts K=96=(c,di); the dj taps are free-dim offsets.
  - Phases are column-tiled in pairs (s, s+8): two concurrent M=64 matmuls
    at tile_position (0,0)/(0,64) fill all 128 PE columns, accumulating
    3 dj taps into one PSUM bank [128part=(half,oc), 32h, 16wg].
  - Evacuate with alternating DVE/ACT f32->bf16 copies into [128, 32, 16, 8]
    staging (slot = pair index); contiguous 8KB-per-partition DMAs out.
  - Host reassembles w = 16*wg + 8*half + slot and upcasts to f32.
"""

import sys

import numpy as np

for _p in ("/opt/trn_rl_repo",):
    if _p not in sys.path:
        sys.path.insert(0, _p)

N, C, H, W = 4, 32, 256, 256
OC = 64
SPE = 16
KDIM = 96  # (c, di) contraction partitions
NCORES = 8
ROWS = H * N // NCORES  # 128 rows per core
GR = 32  # rows per group
GROUPS = ROWS // GR
NPAIR = SPE // 2  # 8 phase pairs (s, s+8)

_cache = {}


def build_nc():
    import bass_rust
    from concourse import bacc, bass, mybir, tile

    f32 = mybir.dt.float32
    bf16 = mybir.dt.bfloat16

    nc = bacc.Bacc()
    xin = nc.dram_tensor("xin", [C, ROWS + 2, W + 2], bf16, kind="ExternalInput")
    wdr = nc.dram_tensor("w", [KDIM, SPE, 3, OC], bf16, kind="ExternalInput")
    # out partition dim = (half, oc); free = (row, wg, slot); w = 16*wg+8*half+slot
    out = nc.dram_tensor("out", [2 * OC, ROWS, W // 16, NPAIR], bf16, kind="ExternalOutput")

    with tile.TileContext(nc) as tc:
        with (
            tc.tile_pool(name="wp", bufs=1) as wp,
            tc.tile_pool(name="xp", bufs=1) as xp,
            tc.tile_pool(name="op", bufs=2) as op,
            tc.tile_pool(name="pp", bufs=4, space=bass.MemorySpace.PSUM) as pp,
        ):
            w_sb = wp.tile([KDIM, SPE, 3, OC], bf16)
            nc.sync.dma_start(w_sb[:], wdr[:])

            x3s = []
            for g in range(GROUPS):
                x3 = xp.tile([KDIM, GR, W + 2], bf16)
                # partition p = c*3 + di reads xin[c, g*32+di : g*32+di+32, :]
                # outermost AP dim = c (32) -> DMA fans across all 16 engines
                src = xin[:, g * GR : g * GR + GR + 2, :]
                src = src.unsqueeze(1).broadcast_to([C, 3, GR + 2, W + 2])
                src.ap = bass_rust.VecI64Pair(
                    [
                        [(ROWS + 2) * (W + 2), C],
                        [W + 2, 3],
                        [W + 2, GR],
                        [1, W + 2],
                    ]
                )
                nc.sync.dma_start(x3[:], src)
                x3s.append(x3)

            for g in range(GROUPS):
                x3 = x3s[g]
                o_sb = op.tile([2 * OC, GR, W // 16, NPAIR], bf16)
                for sp in range(NPAIR):
                    sA, sB = sp, sp + NPAIR
                    ps = pp.tile([2 * OC, GR, 16], f32, tag="ps")
                    for dj in range(3):
                        nc.tensor.matmul(
                            ps[0:OC],
                            w_sb[:, sA, dj, :],
                            x3[:, :, sA + dj : sA + dj + 241 : 16],
                            start=(dj == 0),
                            stop=(dj == 2),
                            tile_position=(0, 0),
                        )
                        nc.tensor.matmul(
                            ps[OC : 2 * OC],
                            w_sb[:, sB, dj, :],
                            x3[:, :, sB + dj : sB + dj + 241 : 16],
                            start=(dj == 0),
                            stop=(dj == 2),
                            tile_position=(0, OC),
                        )
                    if sp % 2 == 0:
                        nc.vector.tensor_copy(o_sb[:, :, :, sp], ps[:])
                    else:
                        nc.scalar.activation(
                            o_sb[:, :, :, sp], ps[:], mybir.ActivationFunctionType.Copy
                        )
                nc.scalar.dma_start(out[:, g * GR : (g + 1) * GR, :, :], o_sb[:])
    nc.finalize()
    return nc


def shard_inputs(x, weight):
    import ml_dtypes

    bf = np.dtype(ml_dtypes.bfloat16)
    x = np.asarray(x, dtype=np.float32)
    weight = np.asarray(weight, dtype=np.float32)
    xp = np.zeros((N, C, H + 2, W + 2), bf)
    xp[:, :, 1:-1, 1:-1] = x.astype(bf)
    # wh[(c*3+di), s, dj, oc] = weight[s, c*9+di*3+dj, oc]
    wh = np.ascontiguousarray(
        weight.reshape(SPE, C, 3, 3, OC).transpose(1, 2, 0, 3, 4)
    ).reshape(KDIM, SPE, 3, OC).astype(bf)
    in_maps = []
    for k in range(NCORES):
        n, r0 = k // 2, (k % 2) * ROWS
        in_maps.append(
            {"xin": np.ascontiguousarray(xp[n, :, r0 : r0 + ROWS + 2, :]), "w": wh}
        )
    return in_maps


def unshard_outputs(results):
    out = np.empty((N, OC, H, W), np.float32)
    for k in range(NCORES):
        n, r0 = k // 2, (k % 2) * ROWS
        od = np.asarray(results[k]["out"]).astype(np.float32)
        od = od.reshape(2, OC, ROWS, W // 16, NPAIR)
        # w = 16*wg + 8*half + slot  ->  (oc, row, wg, half, slot)
        out[n, :, r0 : r0 + ROWS, :] = od.transpose(1, 2, 3, 0, 4).reshape(
            OC, ROWS, W
        )
    return out


def run(x, weight, **spmd_kwargs):
    from concourse.bass_utils import run_bass_kernel_spmd

    in_maps = shard_inputs(x, weight)
    if "nc" not in _cache:
        _cache["nc"] = build_nc()
    res = run_bass_kernel_spmd(_cache["nc"], in_maps, list(range(NCORES)), **spmd_kwargs)
    return unshard_outputs(res.results), res


def kernel(x, weight):
    out, _ = run(x, weight)
    return out


# revision 5
# speedup vs baseline: 2.3963x; 1.2020x over previous
"""Trainium2 Bass kernel for ConvMosaic: 3x3 conv (pad 1) where the weight set
depends on output position p%16 == w%16 (column phase).

Strategy (8 NeuronCores, SPMD):
  - Shard over (N, H): core k handles image k//2, row-half k%2 (128 rows).
  - Host pre-pads each core chunk to [C=32, 130, 258] bf16 (halo rows + zero
    cols).  bf16 halves DMA traffic and doubles PE column rate.
  - On-chip: one x tile [(c,di)=96, 128h, 258w] covers all row groups
    (c-major partition order so the staging DMA's outermost AP dim is 32 ->
    16-engine fanout; per-partition reads are one contiguous 66KB block,
    split into 2 row-chunk DMAs for pipelining).  One matmul contracts
    K=96=(c,di); the dj taps are free-dim offsets.
  - Phases are column-tiled in pairs (s, s+8): two concurrent M=64 matmuls
    at tile_position (0,0)/(0,64) fill all 128 PE columns, accumulating
    3 dj taps into one PSUM bank [128part=(half,oc), 32h, 16wg].  Two
    adjacent pairs share a 2-bank PSUM tile so evacuation is one wide copy.
  - Evacuate with alternating DVE/ACT f32->bf16 copies into
    [128, 32, 8 slot, 16 wg] staging (slot-major -> contiguous step-1 dst,
    2x DVE mode); contiguous 8KB-per-partition DMAs out per group.
  - Host reassembles w = 16*wg + 8*half + slot and upcasts to f32.
"""

import sys

import numpy as np

for _p in ("/opt/trn_rl_repo",):
    if _p not in sys.path:
        sys.path.insert(0, _p)

N, C, H, W = 4, 32, 256, 256
OC = 64
SPE = 16
KDIM = 96  # (c, di) contraction partitions
NCORES = 8
ROWS = H * N // NCORES  # 128 rows per core
GR = 32  # rows per group
GROUPS = ROWS // GR
NPAIR = SPE // 2  # 8 phase pairs (s, s+8)

_cache = {}


def build_nc():
    import bass_rust
    from concourse import bacc, bass, mybir, tile

    f32 = mybir.dt.float32
    bf16 = mybir.dt.bfloat16

    nc = bacc.Bacc()
    xin = nc.dram_tensor("xin", [C, ROWS + 2, W + 2], bf16, kind="ExternalInput")
    wdr = nc.dram_tensor("w", [KDIM, SPE, 3, OC], bf16, kind="ExternalInput")
    # out partition dim = (half, oc); free = (row, slot, wg); w = 16*wg+8*half+slot
    out = nc.dram_tensor("out", [2 * OC, ROWS, NPAIR, W // 16], bf16, kind="ExternalOutput")

    with tile.TileContext(nc) as tc:
        with (
            tc.tile_pool(name="wp", bufs=1) as wp,
            tc.tile_pool(name="xp", bufs=1) as xp,
            tc.tile_pool(name="op", bufs=2) as op,
            tc.tile_pool(name="pp", bufs=3, space=bass.MemorySpace.PSUM) as pp,
        ):
            w_sb = wp.tile([KDIM, SPE, 3, OC], bf16)
            nc.sync.dma_start(w_sb[:], wdr[:])

            # partition p = c*3 + di holds xin[c, di : di+ROWS, :]; group g's
            # 32-row window sits at free offset 32g.  Two row-chunk DMAs so
            # compute on groups 0-1 overlaps the second chunk's load.
            x3 = xp.tile([KDIM, ROWS, W + 2], bf16)
            for half in range(2):
                r0 = half * (ROWS // 2)
                src = xin[:, r0 : r0 + ROWS // 2 + 2, :]
                src = src.unsqueeze(1).broadcast_to([C, 3, ROWS // 2 + 2, W + 2])
                src.ap = bass_rust.VecI64Pair(
                    [
                        [(ROWS + 2) * (W + 2), C],
                        [W + 2, 3],
                        [W + 2, ROWS // 2],
                        [1, W + 2],
                    ]
                )
                nc.sync.dma_start(x3[:, r0 : r0 + ROWS // 2, :], src)

            for g in range(GROUPS):
                o_sb = op.tile([2 * OC, GR, NPAIR, 16], bf16)
                for spp in range(NPAIR // 2):
                    ps = pp.tile([2 * OC, 2, GR, 16], f32, tag="ps")
                    for b in range(2):
                        sA = 2 * spp + b
                        sB = sA + NPAIR
                        for dj in range(3):
                            nc.tensor.matmul(
                                ps[0:OC, b],
                                w_sb[:, sA, dj, :],
                                x3[:, g * GR : (g + 1) * GR, sA + dj : sA + dj + 241 : 16],
                                start=(dj == 0),
                                stop=(dj == 2),
                                tile_position=(0, 0),
                            )
                            nc.tensor.matmul(
                                ps[OC : 2 * OC, b],
                                w_sb[:, sB, dj, :],
                                x3[:, g * GR : (g + 1) * GR, sB + dj : sB + dj + 241 : 16],
                                start=(dj == 0),
                                stop=(dj == 2),
                                tile_position=(0, OC),
                            )
                    # dst iterated (slot, h, wg) to match the contiguous psum
                    # src; innermost 16 contiguous bf16 -> 2x DVE mode
                    dst = o_sb[:, :, 2 * spp : 2 * spp + 2, :]
                    dst = dst.transpose([0, 2, 1, 3])
                    if spp % 2 == 0:
                        nc.vector.tensor_copy(dst, ps[:])
                    else:
                        nc.scalar.activation(
                            dst, ps[:], mybir.ActivationFunctionType.Copy
                        )
                nc.scalar.dma_start(out[:, g * GR : (g + 1) * GR, :, :], o_sb[:])
    nc.finalize()
    return nc


def shard_inputs(x, weight):
    import ml_dtypes

    bf = np.dtype(ml_dtypes.bfloat16)
    x = np.asarray(x, dtype=np.float32)
    weight = np.asarray(weight, dtype=np.float32)
    xp = np.zeros((N, C, H + 2, W + 2), bf)
    xp[:, :, 1:-1, 1:-1] = x.astype(bf)
    # wh[(c*3+di), s, dj, oc] = weight[s, c*9+di*3+dj, oc]
    wh = np.ascontiguousarray(
        weight.reshape(SPE, C, 3, 3, OC).transpose(1, 2, 0, 3, 4)
    ).reshape(KDIM, SPE, 3, OC).astype(bf)
    in_maps = []
    for k in range(NCORES):
        n, r0 = k // 2, (k % 2) * ROWS
        in_maps.append(
            {"xin": np.ascontiguousarray(xp[n, :, r0 : r0 + ROWS + 2, :]), "w": wh}
        )
    return in_maps


def unshard_outputs(results):
    out = np.empty((N, OC, H, W), np.float32)
    for k in range(NCORES):
        n, r0 = k // 2, (k % 2) * ROWS
        od = np.asarray(results[k]["out"]).astype(np.float32)
        od = od.reshape(2, OC, ROWS, NPAIR, W // 16)
        # w = 16*wg + 8*half + slot  ->  order (oc, row, wg, half, slot)
        out[n, :, r0 : r0 + ROWS, :] = od.transpose(1, 2, 4, 0, 3).reshape(
            OC, ROWS, W
        )
    return out


def run(x, weight, **spmd_kwargs):
    from concourse.bass_utils import run_bass_kernel_spmd

    in_maps = shard_inputs(x, weight)
    if "nc" not in _cache:
        _cache["nc"] = build_nc()
    res = run_bass_kernel_spmd(_cache["nc"], in_maps, list(range(NCORES)), **spmd_kwargs)
    return unshard_outputs(res.results), res


def kernel(x, weight):
    out, _ = run(x, weight)
    return out


# revision 7
# speedup vs baseline: 4.6804x; 1.9532x over previous
"""Trainium2 Bass kernel for ConvMosaic: 3x3 conv (pad 1) where the weight set
depends on output position p%16 == w%16 (column phase).

Strategy (8 NeuronCores, SPMD):
  - Shard over (N, H): core k handles image k//2, row-half k%2 (128 rows).
  - Host pre-builds a phase-deinterleaved, dj-replicated bf16 image per core:
      xdev[(dj*32+c), s, h', wg] = xpad[c, h', 16*wg + s + dj]
    (xpad zero-padded to [32, 130, 258]).  Plane s holds exactly the moving
    data for output phase s with the dj column-taps baked into the partition
    dim, so one matmul contracts K=96=(dj,c) with a fully CONTIGUOUS
    512-element moving slice (h-block x 16 wg), and the di row-taps are
    free-dim h offsets accumulated over 3 matmuls.
  - Phases are column-tiled in pairs (2i, 2i+1): two concurrent M=64 matmuls
    at tile_position (0,0)/(0,64) fill all 128 PE columns; two adjacent
    pairs share a 2-bank PSUM tile [128part=(half,oc), 2, 32h, 16wg].
  - Input DMA is fully linear (one plane-pair chunk per pair of phases,
    66KB contiguous per partition total) and c-major -> 16-engine fanout.
  - Evacuate with alternating DVE/ACT f32->bf16 copies into
    [128, 32, 8 slot, 16 wg] staging (contiguous step-1 dst); contiguous
    8KB-per-partition DMAs out per row group.
  - Host reassembles w = 16*wg + 2*slot + half and upcasts to f32.
"""

import sys

import numpy as np

for _p in ("/opt/trn_rl_repo",):
    if _p not in sys.path:
        sys.path.insert(0, _p)

N, C, H, W = 4, 32, 256, 256
OC = 64
SPE = 16
KDIM = 96  # (dj, c) contraction partitions
NCORES = 8
ROWS = H * N // NCORES  # 128 rows per core
GR = 32  # rows per group
GROUPS = ROWS // GR
NPAIR = SPE // 2  # 8 phase pairs (2i, 2i+1)
WG = W // 16  # 16 column groups

_cache = {}


def build_nc():
    from concourse import bacc, bass, mybir, tile

    f32 = mybir.dt.float32
    bf16 = mybir.dt.bfloat16

    nc = bacc.Bacc()
    xin = nc.dram_tensor("xin", [KDIM, SPE, ROWS + 2, WG], bf16, kind="ExternalInput")
    wdr = nc.dram_tensor("w", [KDIM, SPE, 3, OC], bf16, kind="ExternalInput")
    # out partition dim = (half, oc); free = (row, slot, wg); w = 16*wg+2*slot+half
    out = nc.dram_tensor("out", [2 * OC, ROWS, NPAIR, WG], bf16, kind="ExternalOutput")

    with tile.TileContext(nc) as tc:
        with (
            tc.tile_pool(name="wp", bufs=1) as wp,
            tc.tile_pool(name="xp", bufs=1) as xp,
            tc.tile_pool(name="op", bufs=2) as op,
            tc.tile_pool(name="pp", bufs=3, space=bass.MemorySpace.PSUM) as pp,
        ):
            w_sb = wp.tile([KDIM, SPE, 3, OC], bf16)
            nc.sync.dma_start(w_sb[:], wdr[:])

            # one plane-pair chunk per phase pair -> input overlaps compute
            x_sb = xp.tile([KDIM, SPE, ROWS + 2, WG], bf16)
            for i in range(NPAIR):
                nc.sync.dma_start(
                    x_sb[:, 2 * i : 2 * i + 2], xin[:, 2 * i : 2 * i + 2]
                )

            for g in range(GROUPS):
                o_sb = op.tile([2 * OC, GR, NPAIR, WG], bf16)
                for pp_i in range(NPAIR // 2):
                    ps = pp.tile([2 * OC, 2, GR, WG], f32, tag="ps")
                    for b in range(2):
                        i = 2 * pp_i + b
                        sA, sB = 2 * i, 2 * i + 1
                        for di in range(3):
                            nc.tensor.matmul(
                                ps[0:OC, b],
                                w_sb[:, sA, di, :],
                                x_sb[:, sA, g * GR + di : g * GR + di + GR, :],
                                start=(di == 0),
                                stop=(di == 2),
                                tile_position=(0, 0),
                            )
                            nc.tensor.matmul(
                                ps[OC : 2 * OC, b],
                                w_sb[:, sB, di, :],
                                x_sb[:, sB, g * GR + di : g * GR + di + GR, :],
                                start=(di == 0),
                                stop=(di == 2),
                                tile_position=(0, OC),
                            )
                    # dst iterated (slot, h, wg) to match the contiguous psum
                    # src; innermost 16 contiguous bf16 -> 2x DVE mode
                    dst = o_sb[:, :, 2 * pp_i : 2 * pp_i + 2, :]
                    dst = dst.transpose([0, 2, 1, 3])
                    if pp_i % 2 == 0:
                        nc.vector.tensor_copy(dst, ps[:])
                    else:
                        nc.scalar.activation(
                            dst, ps[:], mybir.ActivationFunctionType.Copy
                        )
                nc.scalar.dma_start(out[:, g * GR : (g + 1) * GR, :, :], o_sb[:])
    nc.finalize()
    return nc


def shard_inputs(x, weight):
    import ml_dtypes

    bf = np.dtype(ml_dtypes.bfloat16)
    x = np.asarray(x, dtype=np.float32)
    weight = np.asarray(weight, dtype=np.float32)
    xpad = np.zeros((N, C, H + 2, W + 2), np.float32)
    xpad[:, :, 1:-1, 1:-1] = x
    # xdev[n, dj, c, s, h', wg] = xpad[n, c, h', 16*wg + s + dj]
    # strided view: xpad[..., 16*wg + (s+dj)] with o = s+dj in 0..17
    xv = np.lib.stride_tricks.sliding_window_view(xpad, 18, axis=3)
    # xv[n, c, h', 16*wg, o]; take wg steps of 16
    xv = xv[:, :, :, ::16, :]  # [N, C, 130, 16wg, 18o]
    xdev = np.empty((N, 3, C, SPE, H + 2, WG), bf)
    for dj in range(3):
        # planes s = o - dj; o = s + dj in [dj, dj+16)
        xdev[:, dj] = xv[:, :, :, :, dj : dj + SPE].transpose(0, 1, 4, 2, 3).astype(bf)
    # wh[(dj*32+c), s, di, oc] = weight[s, c*9+di*3+dj, oc]
    wh = np.ascontiguousarray(
        weight.reshape(SPE, C, 3, 3, OC).transpose(3, 1, 0, 2, 4)
    ).reshape(3 * C, SPE, 3, OC).astype(bf)
    in_maps = []
    for k in range(NCORES):
        n, r0 = k // 2, (k % 2) * ROWS
        xc = np.ascontiguousarray(
            xdev[n, :, :, :, r0 : r0 + ROWS + 2, :].reshape(KDIM, SPE, ROWS + 2, WG)
        )
        in_maps.append({"xin": xc, "w": wh})
    return in_maps


def unshard_outputs(results):
    out = np.empty((N, OC, H, W), np.float32)
    for k in range(NCORES):
        n, r0 = k // 2, (k % 2) * ROWS
        od = np.asarray(results[k]["out"]).astype(np.float32)
        od = od.reshape(2, OC, ROWS, NPAIR, WG)
        # w = 16*wg + 2*slot + half  ->  order (oc, row, wg, slot, half)
        out[n, :, r0 : r0 + ROWS, :] = od.transpose(1, 2, 4, 3, 0).reshape(
            OC, ROWS, W
        )
    return out


def run(x, weight, **spmd_kwargs):
    from concourse.bass_utils import run_bass_kernel_spmd

    in_maps = shard_inputs(x, weight)
    if "nc" not in _cache:
        _cache["nc"] = build_nc()
    res = run_bass_kernel_spmd(_cache["nc"], in_maps, list(range(NCORES)), **spmd_kwargs)
    return unshard_outputs(res.results), res


def kernel(x, weight):
    out, _ = run(x, weight)
    return out


# revision 9
# speedup vs baseline: 5.0983x; 1.0893x over previous
"""Trainium2 Bass kernel for ConvMosaic: 3x3 conv (pad 1) where the weight set
depends on output position p%16 == w%16 (column phase).

Strategy (8 NeuronCores, SPMD):
  - Shard over (N, H): core k handles image k//2, row-half k%2 (128 rows).
  - Host pre-builds a phase-deinterleaved, dj-replicated bf16 image per core:
      xdev[(dj*32+c), s, h', wg] = xpad[c, h', 16*wg + s + dj]
    (xpad zero-padded to [32, 130, 258]).  Plane s holds exactly the moving
    data for output phase s with the dj column-taps baked into the partition
    dim, so one matmul contracts K=96=(dj,c) with a fully CONTIGUOUS
    512-element moving slice (32h x 16wg), and the di row-taps are free-dim
    h offsets accumulated over 3 matmuls.
  - Phase-pair-outer pipeline: pair i=(2i, 2i+1) needs only input planes
    (2i, 2i+1); it computes all 4 row groups (two concurrent M=64 matmuls at
    tile_position (0,0)/(0,64) fill all 128 PE columns; loop di -> g so each
    stationary is loaded once per di and reused for 4 groups -- the two
    column-tile halves hold weights in separate subarray strips), evacuates
    each group's PSUM bank with alternating DVE/ACT f32->bf16 copies into a
    per-pair [128, ROWS, WG] staging tile, and DMAs that block out
    immediately -- so output DMA overlaps the remaining input stream.
  - Weights load on the scalar HWDGE ring in parallel with input chunk 0.
  - Host reassembles w = 16*wg + 2*i + half and upcasts to f32.
"""

import sys

import numpy as np

for _p in ("/opt/trn_rl_repo",):
    if _p not in sys.path:
        sys.path.insert(0, _p)

N, C, H, W = 4, 32, 256, 256
OC = 64
SPE = 16
KDIM = 96  # (dj, c) contraction partitions
NCORES = 8
ROWS = H * N // NCORES  # 128 rows per core
GR = 32  # rows per group
GROUPS = ROWS // GR
NPAIR = SPE // 2  # 8 phase pairs (2i, 2i+1)
WG = W // 16  # 16 column groups

_cache = {}


def build_nc():
    from concourse import bacc, bass, mybir, tile

    f32 = mybir.dt.float32
    bf16 = mybir.dt.bfloat16

    nc = bacc.Bacc()
    xin = nc.dram_tensor("xin", [KDIM, SPE, ROWS + 2, WG], bf16, kind="ExternalInput")
    wdr = nc.dram_tensor("w", [KDIM, SPE, 3, OC], bf16, kind="ExternalInput")
    # out partition dim = (half, oc); free = (pair, row, wg); w = 16*wg+2*pair+half
    out = nc.dram_tensor("out", [2 * OC, NPAIR, ROWS, WG], bf16, kind="ExternalOutput")

    with tile.TileContext(nc) as tc:
        with (
            tc.tile_pool(name="wp", bufs=1) as wp,
            tc.tile_pool(name="xp", bufs=1) as xp,
            tc.tile_pool(name="op", bufs=3) as op,
            tc.tile_pool(name="pp", bufs=8, space=bass.MemorySpace.PSUM) as pp,
        ):
            w_sb = wp.tile([KDIM, SPE, 3, OC], bf16)
            nc.scalar.dma_start(w_sb[:], wdr[:])

            # one 2-plane chunk per phase pair -> input overlaps compute
            x_sb = xp.tile([KDIM, SPE, ROWS + 2, WG], bf16)
            for i in range(NPAIR):
                nc.sync.dma_start(
                    x_sb[:, 2 * i : 2 * i + 2], xin[:, 2 * i : 2 * i + 2]
                )

            for i in range(NPAIR):
                sA, sB = 2 * i, 2 * i + 1
                o_sb = op.tile([2 * OC, ROWS, WG], bf16)
                psl = [
                    pp.tile([2 * OC, GR, WG], f32, tag="ps", name=f"ps_{i}_{g}")
                    for g in range(GROUPS)
                ]
                for di in range(3):
                    for g in range(GROUPS):
                        nc.tensor.matmul(
                            psl[g][0:OC],
                            w_sb[:, sA, di, :],
                            x_sb[:, sA, g * GR + di : g * GR + di + GR, :],
                            start=(di == 0),
                            stop=(di == 2),
                            tile_position=(0, 0),
                        )
                        nc.tensor.matmul(
                            psl[g][OC : 2 * OC],
                            w_sb[:, sB, di, :],
                            x_sb[:, sB, g * GR + di : g * GR + di + GR, :],
                            start=(di == 0),
                            stop=(di == 2),
                            tile_position=(0, OC),
                        )
                for g in range(GROUPS):
                    dst = o_sb[:, g * GR : (g + 1) * GR, :]
                    if g % 2 == 0:
                        nc.vector.tensor_copy(dst, psl[g][:])
                    else:
                        nc.scalar.activation(
                            dst, psl[g][:], mybir.ActivationFunctionType.Copy
                        )
                nc.scalar.dma_start(out[:, i, :, :], o_sb[:])
    nc.finalize()
    return nc


def shard_inputs(x, weight):
    import ml_dtypes

    bf = np.dtype(ml_dtypes.bfloat16)
    x = np.asarray(x, dtype=np.float32)
    weight = np.asarray(weight, dtype=np.float32)
    xpad = np.zeros((N, C, H + 2, W + 2), np.float32)
    xpad[:, :, 1:-1, 1:-1] = x
    # xdev[n, dj, c, s, h', wg] = xpad[n, c, h', 16*wg + s + dj]
    xv = np.lib.stride_tricks.sliding_window_view(xpad, 18, axis=3)
    xv = xv[:, :, :, ::16, :]  # [N, C, H+2, wg, o] with o = s+dj
    xdev = np.empty((N, 3, C, SPE, H + 2, WG), bf)
    for dj in range(3):
        xdev[:, dj] = xv[:, :, :, :, dj : dj + SPE].transpose(0, 1, 4, 2, 3).astype(bf)
    # wh[(dj*32+c), s, di, oc] = weight[s, c*9+di*3+dj, oc]
    wh = np.ascontiguousarray(
        weight.reshape(SPE, C, 3, 3, OC).transpose(3, 1, 0, 2, 4)
    ).reshape(3 * C, SPE, 3, OC).astype(bf)
    in_maps = []
    for k in range(NCORES):
        n, r0 = k // 2, (k % 2) * ROWS
        xc = np.ascontiguousarray(
            xdev[n, :, :, :, r0 : r0 + ROWS + 2, :].reshape(KDIM, SPE, ROWS + 2, WG)
        )
        in_maps.append({"xin": xc, "w": wh})
    return in_maps


def unshard_outputs(results):
    out = np.empty((N, OC, H, W), np.float32)
    for k in range(NCORES):
        n, r0 = k // 2, (k % 2) * ROWS
        od = np.asarray(results[k]["out"]).astype(np.float32)
        od = od.reshape(2, OC, NPAIR, ROWS, WG)
        # w = 16*wg + 2*pair + half  ->  order (oc, row, wg, pair, half)
        out[n, :, r0 : r0 + ROWS, :] = od.transpose(1, 3, 4, 2, 0).reshape(
            OC, ROWS, W
        )
    return out


def run(x, weight, **spmd_kwargs):
    from concourse.bass_utils import run_bass_kernel_spmd

    in_maps = shard_inputs(x, weight)
    if "nc" not in _cache:
        _cache["nc"] = build_nc()
    res = run_bass_kernel_spmd(_cache["nc"], in_maps, list(range(NCORES)), **spmd_kwargs)
    return unshard_outputs(res.results), res


def kernel(x, weight):
    out, _ = run(x, weight)
    return out


# revision 10
# speedup vs baseline: 6.7258x; 1.3192x over previous
"""Trainium2 Bass kernel for ConvMosaic: 3x3 conv (pad 1) where the weight set
depends on output position p%16 == w%16 (column phase).

Strategy (8 NeuronCores, SPMD):
  - Shard over (N, H): core k handles image k//2, row-half k%2 (128 rows).
  - Host pre-builds a phase-deinterleaved bf16 image per core, partitioned
    by ROW GROUP (no dj replication -- input is only ~2.5MB/core):
      x_sb[(g*32+c), o, hh, wg] = xpad[c, g*32+hh, 16*wg + o]
    for o in 0..17, hh in 0..33 (32-row group + 2 halo rows).
  - The 128x128 PE array is row-tiled into four 32-row strips (strip = row
    group g) x two 64-col halves (phases 2i, 2i+1): per tap t=(di,dj) of
    phase s, strip g runs a K=32 matmul with tile_position (32g, 0|64),
    moving slice x_sb[32g:32g+32, s+dj, di:di+32, :] (contiguous 512 elems),
    accumulating 9 taps into PSUM bank g [128=(half,oc), 32h, 16wg].
    8 concurrent matmuls per tap slot -> full PE column utilization; each
    strip's LDWEIGHTS overlaps other strips' matmuls (disjoint row groups).
  - Weights are host-replicated per strip: w_sb[(g*32+c), s, t, oc],
    DMAed in per-pair chunks on the scalar ring (parallel with input).
  - Pair-outer pipeline: pair i needs input planes 2i..2i+3 only; computes
    all 4 strips, evacuates banks with alternating DVE/ACT f32->bf16 copies
    into per-pair [128, ROWS, WG] staging, DMAs out immediately (2-pair
    blocks for bigger packets).
  - Host reassembles w = 16*wg + 2*i + half and upcasts to f32.
"""

import sys

import numpy as np

for _p in ("/opt/trn_rl_repo",):
    if _p not in sys.path:
        sys.path.insert(0, _p)

N, C, H, W = 4, 32, 256, 256
OC = 64
SPE = 16
NCORES = 8
ROWS = H * N // NCORES  # 128 rows per core
GR = 32  # rows per group/strip
GROUPS = ROWS // GR
NPAIR = SPE // 2  # 8 phase pairs (2i, 2i+1)
WG = W // 16  # 16 column groups
NPL = 18  # deinterleave planes o = s + dj
HH = GR + 2  # 34 rows per strip incl halo

_cache = {}


def build_nc():
    from concourse import bacc, bass, mybir, tile

    f32 = mybir.dt.float32
    bf16 = mybir.dt.bfloat16

    nc = bacc.Bacc()
    xin = nc.dram_tensor("xin", [4 * C, NPL, HH, WG], bf16, kind="ExternalInput")
    wdr = nc.dram_tensor("w", [4 * C, SPE, 9, OC], bf16, kind="ExternalInput")
    # out partition dim = (half, oc); free = (pair, row, wg); w = 16*wg+2*pair+half
    out = nc.dram_tensor("out", [2 * OC, NPAIR, ROWS, WG], bf16, kind="ExternalOutput")

    with tile.TileContext(nc) as tc:
        with (
            tc.tile_pool(name="wp", bufs=1) as wp,
            tc.tile_pool(name="xp", bufs=1) as xp,
            tc.tile_pool(name="op", bufs=2) as op,
            tc.tile_pool(name="pp", bufs=8, space=bass.MemorySpace.PSUM) as pp,
        ):
            w_sb = wp.tile([4 * C, SPE, 9, OC], bf16)
            for i in range(NPAIR):
                nc.scalar.dma_start(w_sb[:, 2 * i : 2 * i + 2], wdr[:, 2 * i : 2 * i + 2])

            # plane chunks 0-5 / 6-11 / 12-17 gate pairs (0,1) / (2,3,4) / (5,6,7)
            x_sb = xp.tile([4 * C, NPL, HH, WG], bf16)
            for ch in range(3):
                nc.sync.dma_start(
                    x_sb[:, 6 * ch : 6 * ch + 6], xin[:, 6 * ch : 6 * ch + 6]
                )

            for ii in range(NPAIR // 2):
                o_sb = op.tile([2 * OC, 2, ROWS, WG], bf16)
                for b in range(2):
                    i = 2 * ii + b
                    sA, sB = 2 * i, 2 * i + 1
                    psl = [
                        pp.tile([2 * OC, GR, WG], f32, tag="ps", name=f"ps_{i}_{g}")
                        for g in range(GROUPS)
                    ]
                    for t in range(9):
                        di, dj = t // 3, t % 3
                        for g in range(GROUPS):
                            nc.tensor.matmul(
                                psl[g][0:OC],
                                w_sb[g * C : (g + 1) * C, sA, t, :],
                                x_sb[g * C : (g + 1) * C, sA + dj, di : di + GR, :],
                                start=(t == 0),
                                stop=(t == 8),
                                tile_position=(g * C, 0),
                            )
                            nc.tensor.matmul(
                                psl[g][OC : 2 * OC],
                                w_sb[g * C : (g + 1) * C, sB, t, :],
                                x_sb[g * C : (g + 1) * C, sB + dj, di : di + GR, :],
                                start=(t == 0),
                                stop=(t == 8),
                                tile_position=(g * C, OC),
                            )
                    for g in range(GROUPS):
                        dst = o_sb[:, b, g * GR : (g + 1) * GR, :]
                        if g % 2 == 0:
                            nc.vector.tensor_copy(dst, psl[g][:])
                        else:
                            nc.scalar.activation(
                                dst, psl[g][:], mybir.ActivationFunctionType.Copy
                            )
                nc.scalar.dma_start(out[:, 2 * ii : 2 * ii + 2, :, :], o_sb[:])
    nc.finalize()
    return nc


def shard_inputs(x, weight):
    import ml_dtypes

    bf = np.dtype(ml_dtypes.bfloat16)
    x = np.asarray(x, dtype=np.float32)
    weight = np.asarray(weight, dtype=np.float32)
    xpad = np.zeros((N, C, H + 2, W + 2), np.float32)
    xpad[:, :, 1:-1, 1:-1] = x
    # xv[n, c, h', wg, o] = xpad[n, c, h', 16*wg + o], o in 0..17
    xv = np.lib.stride_tricks.sliding_window_view(xpad, NPL, axis=3)[:, :, :, ::16, :]
    xv = xv.astype(bf)
    # wh[(g*32+c), s, t, oc] = weight[s, c*9+t, oc], replicated over g
    wh1 = np.ascontiguousarray(
        weight.reshape(SPE, C, 9, OC).transpose(1, 0, 2, 3)
    ).astype(bf)  # [C, SPE, 9, OC]
    wh = np.ascontiguousarray(np.broadcast_to(wh1, (4, C, SPE, 9, OC))).reshape(
        4 * C, SPE, 9, OC
    )
    in_maps = []
    for k in range(NCORES):
        n, r0 = k // 2, (k % 2) * ROWS
        xc = np.empty((GROUPS, C, NPL, HH, WG), bf)
        for g in range(GROUPS):
            blk = xv[n, :, r0 + g * GR : r0 + g * GR + HH, :, :]  # [C, HH, WG, NPL]
            xc[g] = blk.transpose(0, 3, 1, 2)
        in_maps.append({"xin": xc.reshape(4 * C, NPL, HH, WG), "w": wh})
    return in_maps


def unshard_outputs(results):
    out = np.empty((N, OC, H, W), np.float32)
    for k in range(NCORES):
        n, r0 = k // 2, (k % 2) * ROWS
        od = np.asarray(results[k]["out"]).astype(np.float32)
        od = od.reshape(2, OC, NPAIR, ROWS, WG)
        # w = 16*wg + 2*pair + half  ->  order (oc, row, wg, pair, half)
        out[n, :, r0 : r0 + ROWS, :] = od.transpose(1, 3, 4, 2, 0).reshape(
            OC, ROWS, W
        )
    return out


def run(x, weight, **spmd_kwargs):
    from concourse.bass_utils import run_bass_kernel_spmd

    in_maps = shard_inputs(x, weight)
    if "nc" not in _cache:
        _cache["nc"] = build_nc()
    res = run_bass_kernel_spmd(_cache["nc"], in_maps, list(range(NCORES)), **spmd_kwargs)
    return unshard_outputs(res.results), res


def kernel(x, weight):
    out, _ = run(x, weight)
    return out
